# revision 1
# baseline (speedup 1.0000x reference)
"""AGDNConv (3-hop attention diffusion GNN) on 8 trn2 NeuronCores.

Sharding: edges partitioned by dst-owner (owner = dst // 12544); node tables
replicated or AllGathered per hop. Per-core segment sums use a degree-class
slot layout so they become strided tensor_reduce ops: a node of in-degree d
gets C = next-class(d) contiguous edge slots on one partition; reducing over
the class axis yields the per-node sum. The instruction stream is identical
on all cores (SPMD); per-core variability lives in index/mask input tensors.
Attention softmax uses the max-free identity
  a[e] = exp(e_e) / sqrt(s_dst[dst_e] * s_src[src_e]).
"""
import sys
sys.path.insert(0, "/opt/trn_rl_repo")
import os
import numpy as np
import ml_dtypes

USE_COLL = os.environ.get("AGDN_NOCOLL", "") != "1"
STAGE = int(os.environ.get("AGDN_STAGE", "8"))
P = 128
N = 100000
IN = 128
H = 3
D = 16
HD = 48
K = 3
NEG = 0.2
EPS = 1e-9
NCORES = 8
NS = 12544
NP_ = NCORES * NS
CLASSES = [4, 8, 16, 24, 32, 48, 64, 96, 128]


# ---------------------------------------------------------------- host prep
def _pack_side(key_node, other_node, n_lo):
    loc = (key_node - n_lo).astype(np.int64)
    order = np.argsort(loc, kind="stable")
    loc_s = loc[order]
    other_s = other_node[order]
    deg = np.bincount(loc_s, minlength=NS)
    assert deg.max() <= CLASSES[-1], f"degree {deg.max()} exceeds max class"
    starts = np.concatenate([[0], np.cumsum(deg)[:-1]])
    cls_of = np.full(NS, -1, np.int64)
    lo = 0
    for ci, C in enumerate(CLASSES):
        cls_of[(deg > lo) & (deg <= C)] = ci
        lo = C
    members = [np.where(cls_of == ci)[0] for ci in range(len(CLASSES))]
    zeros = np.where(deg == 0)[0]
    return dict(members=members, zeros=zeros, deg=deg, starts=starts,
                other_s=other_s)


def _layout(counts_max, gz_max):
    G, vg0, je0, plan = [], [], [], []
    v, j = 0, 0
    for ci, C in enumerate(CLASSES):
        g = int(np.ceil(counts_max[ci] / P))
        G.append(g)
        vg0.append(v)
        je0.append(j)
        cols = g * C
        step = max(C, (64 // C) * C)
        s = 0
        while s < cols:
            w = min(step, cols - s)
            plan.append((j + s, w, C))
            s += w
        v += g
        j += g * C
    vg0.append(v)
    v += int(np.ceil(gz_max / P))
    return G, vg0, je0, max(v, 1), max(((j + 3) // 4) * 4, 4), plan


def _fill_core(pack, G, vg0, je0, NV, NTE, n_lo):
    ioth = np.full((P, NTE), N, np.int32)
    ikey = np.full((P, NTE), N, np.int32)
    mask = np.zeros((P, NTE, 4), np.float32)
    vrow = np.full(NS, -1, np.int64)
    deg, starts, other_s = pack["deg"], pack["starts"], pack["other_s"]
    for ci, C in enumerate(CLASSES):
        mem = pack["members"][ci]
        g_c = max(G[ci], 1)
        for i, nl in enumerate(mem):
            g, p = i % g_c, i // g_c
            vrow[nl] = p * NV + vg0[ci] + g
            d, s0 = deg[nl], starts[nl]
            je = je0[ci] + g * C
            ioth[p, je:je + d] = other_s[s0:s0 + d]
            ikey[p, je:je + d] = n_lo + nl
            mask[p, je:je + d, 0:3] = 1.0
    mem = pack["zeros"]
    gz = max(int(np.ceil(len(mem) / P)), 1)
    for i, nl in enumerate(mem):
        vrow[nl] = (i // gz) * NV + vg0[len(CLASSES)] + i % gz
    return ioth, ikey, mask, vrow


def host_prep(src, dst):
    sides = {}
    for side, key, oth in (("d", dst, src), ("s", src, dst)):
        packs = []
        for c in range(NCORES):
            m = (key >= c * NS) & (key < (c + 1) * NS)
            packs.append(_pack_side(key[m], oth[m], c * NS))
        counts_max = np.max(
            np.array([[len(p) for p in pk["members"]] for pk in packs]), axis=0)
        gz_max = max(len(pk["zeros"]) for pk in packs)
        G, vg0, je0, NV, NTE, plan = _layout(counts_max, gz_max)
        cores, vmap = [], np.zeros(NP_ + 1, np.int64)
        for c in range(NCORES):
            ioth, ikey, mask, vrow = _fill_core(
                packs[c], G, vg0, je0, NV, NTE, c * NS)
            cores.append(dict(ioth=ioth, ikey=ikey, mask=mask, vrow=vrow))
            vmap[c * NS:(c + 1) * NS] = c * (P * NV) + vrow
        sides[side] = dict(NV=NV, NTE=NTE, plan=plan, cores=cores, vmap=vmap)
    return sides


def _vg_lookup(plan):
    lk, vg, last_C, cj, cvg = {}, 0, None, None, None
    for (j0, nj, C) in plan:
        if C != last_C:
            cj, cvg, last_C = j0, vg, C
        lk[(j0, C)] = cvg + (j0 - cj) // C
        vg = cvg + (j0 - cj + nj) // C
    return lk


# ---------------------------------------------------------------- device
def build_nc(NVD, NTED, pland, NVS, NTES, plans):
    import concourse.bass as bass
    import concourse.bacc as bacc
    import concourse.mybir as mybir
    import concourse.tile as tile
    f32, bf16, i32 = mybir.dt.float32, mybir.dt.bfloat16, mybir.dt.int32
    AT, AF, AX = mybir.AluOpType, mybir.ActivationFunctionType, mybir.AxisListType
    IOA = bass.IndirectOffsetOnAxis
    NSVD, NSVS = P * NVD, P * NVS
    lk_d, lk_s = _vg_lookup(pland), _vg_lookup(plans)
    MNJ = max(max(nj for (_, nj, _) in pland), max(nj for (_, nj, _) in plans))

    nc = bacc.Bacc("TRN2", target_bir_lowering=False, debug=False,
                   num_devices=NCORES)
    featT = nc.dram_tensor("featT", [P, NP_], bf16, kind="ExternalInput")
    W_in = nc.dram_tensor("W_in", [P, HD], f32, kind="ExternalInput")
    attn_lr = nc.dram_tensor("attn_lr", [P, 2 * HD], f32, kind="ExternalInput")
    hop_lr = nc.dram_tensor("hop_lr", [P, 2 * HD], f32, kind="ExternalInput")
    scales4 = nc.dram_tensor("scales4", [P, (K + 1) * HD], f32, kind="ExternalInput")
    offpos4 = nc.dram_tensor("offpos4", [P, (K + 1) * HD], f32, kind="ExternalInput")
    bias_in = nc.dram_tensor("bias_in", [P, HD], f32, kind="ExternalInput")
    iSRCd = nc.dram_tensor("iSRCd", [P, NTED], i32, kind="ExternalInput")
    iDSTd = nc.dram_tensor("iDSTd", [P, NTED], i32, kind="ExternalInput")
    iSDv = nc.dram_tensor("iSDv", [P, NTED], i32, kind="ExternalInput")
    iSSv = nc.dram_tensor("iSSv", [P, NTED], i32, kind="ExternalInput")
    iCUR = nc.dram_tensor("iCUR", [P, NTED], i32, kind="ExternalInput")
    iSRCs = nc.dram_tensor("iSRCs", [P, NTES], i32, kind="ExternalInput")
    iDSTs = nc.dram_tensor("iDSTs", [P, NTES], i32, kind="ExternalInput")
    iQ = nc.dram_tensor("iQ", [P, NVD], i32, kind="ExternalInput")
    maskA = nc.dram_tensor("maskA", [P, NTED * 4], bf16, kind="ExternalInput")
    maskB = nc.dram_tensor("maskB", [P, NTES * 4], bf16, kind="ExternalInput")
    out_t = nc.dram_tensor("out", [NSVD, HD], f32, kind="ExternalOutput")

    SD = nc.dram_tensor("SD", [NSVD, 16], f32, kind="Internal")
    SSL = nc.dram_tensor("SSL", [NCORES * NSVS, 16], f32, kind="Internal")
    FTELER = nc.dram_tensor("FTELER", [NP_ + 1, 64], f32, kind="Internal")
    SSh = nc.dram_tensor("SSh", [NSVS, 4], f32, kind="Internal")
    SS = nc.dram_tensor("SS", [NCORES * NSVS, 4], f32, kind="Internal",
                        addr_space="Shared")
    CURSH = nc.dram_tensor("CURSH", [NSVD, 64], bf16, kind="Internal")
    CURG = [nc.dram_tensor(f"CURG{k}", [NCORES * NSVD, 64], bf16,
                           kind="Internal", addr_space="Shared")
            for k in range(K - 1)]
    CURGL = [nc.dram_tensor(f"CURGL{k}", [NCORES * NSVD, 64], bf16,
                            kind="Internal") for k in range(K - 1)]
    HSTK = [nc.dram_tensor(f"HSTK{k}", [NSVD, HD], f32, kind="Internal")
            for k in range(K)]
    rg = [list(range(NCORES))]

    with tile.TileContext(nc) as tc:
        with tc.tile_pool(name="persist", bufs=1) as pp, \
             tc.tile_pool(name="work", bufs=1) as wp, \
             tc.tile_pool(name="gat", bufs=2) as gp, \
             tc.tile_pool(name="ps", bufs=2, space="PSUM") as psp:
            # ---- weights / constants ----
            wwa = pp.tile([P, 54], bf16)
            wf = wp.tile([P, HD], f32, tag="wf")
            nc.sync.dma_start(wf[:], W_in.ap())
            alr = pp.tile([P, 2 * HD], f32)
            nc.sync.dma_start(alr[:], attn_lr.ap())
            hlr = pp.tile([P, 2 * HD], f32)
            nc.sync.dma_start(hlr[:], hop_lr.ap())
            sc4 = pp.tile([P, (K + 1) * HD], f32)
            nc.sync.dma_start(sc4[:], scales4.ap())
            op4 = pp.tile([P, (K + 1) * HD], f32)
            nc.sync.dma_start(op4[:], offpos4.ap())
            bia = pp.tile([P, HD], f32)
            nc.sync.dma_start(bia[:], bias_in.ap())
            epst = pp.tile([P, 1], f32)
            nc.vector.memset(epst[:], EPS)
            nc.vector.tensor_copy(wwa[:, 0:HD], wf[:])
            tmp = wp.tile([P, HD], f32, tag="tmp")
            tmp3 = wp.tile([P, 3], f32, tag="tmp3")
            for t in range(2):
                nc.vector.tensor_tensor(
                    out=tmp[:], in0=wf[:],
                    in1=alr[:, t * HD:(t + 1) * HD], op=AT.mult)
                nc.vector.tensor_reduce(
                    out=tmp3[:], in_=tmp[:].rearrange("p (h d) -> p h d", d=D),
                    axis=AX.X, op=AT.add)
                nc.vector.tensor_copy(wwa[:, HD + 3 * t:HD + 3 * t + 3], tmp3[:])

            # ---- P0: replicated feat matmul -> FTELER rows [ft|el|er|pad] ----
            GRP = 1024
            for g in range(NP_ // GRP):
                fch = wp.tile([P, GRP], bf16, tag="fch")
                nc.sync.dma_start(fch[:], featT.ap()[:, g * GRP:(g + 1) * GRP])
                ps = psp.tile([P, 8 * 54], f32, tag="p0ps")
                for t in range(8):
                    nc.tensor.matmul(
                        out=ps[:, t * 54:(t + 1) * 54],
                        lhsT=fch[:, t * P:(t + 1) * P],
                        rhs=wwa[:], start=True, stop=True)
                stg = wp.tile([P, 8 * 64], f32, tag="p0st")
                nc.vector.memset(stg[:], 0.0)
                nc.vector.tensor_copy(
                    stg[:].rearrange("q (t e) -> q t e", e=64)[:, :, 0:54],
                    ps[:].rearrange("q (t e) -> q t e", e=54))
                nc.sync.dma_start(
                    FTELER.ap()[g * GRP:(g + 1) * GRP, :].rearrange(
                        "(t p) e -> p t e", t=8),
                    stg[:].rearrange("q (t e) -> q t e", e=64))
            zr = wp.tile([1, 64], f32, tag="zr")
            nc.vector.memset(zr[:], 0.0)
            nc.sync.dma_start(FTELER.ap()[NP_:NP_ + 1, :], zr[:])

            def idx_chunk(src_dram, j0, nj, tag):
                t = gp.tile([P, MNJ], i32, tag=tag)
                nc.sync.dma_start(t[:, :nj], src_dram.ap()[:, j0:j0 + nj])
                return t

            # ---- score passes ----
            def score_pass(plan, lk, iel_d, ier_d, mk_d, sv, exp_keep):
                for (j0, nj, C) in plan:
                    iel = idx_chunk(iel_d, j0, nj, "iel")
                    ier = idx_chunk(ier_d, j0, nj, "ier")
                    mkc = gp.tile([P, MNJ * 4], bf16, tag="mkc")
                    nc.sync.dma_start(mkc[:, :nj * 4],
                                      mk_d.ap()[:, j0 * 4:(j0 + nj) * 4])
                    g1 = gp.tile([P, MNJ * 6], f32, tag="g6a")
                    g2 = gp.tile([P, MNJ * 6], f32, tag="g6b")
                    nc.gpsimd.indirect_dma_start(
                        out=g1[:, :nj * 6], out_offset=None, in_=FTELER.ap(),
                        in_offset=IOA(ap=iel[:, :nj], axis=0),
                        element_offset=48)
                    nc.gpsimd.indirect_dma_start(
                        out=g2[:, :nj * 6], out_offset=None, in_=FTELER.ap(),
                        in_offset=IOA(ap=ier[:, :nj], axis=0),
                        element_offset=48)
                    if exp_keep is not None:
                        et = exp_keep[:, j0 * 4:(j0 + nj) * 4]
                    else:
                        ett = gp.tile([P, MNJ * 4], bf16, tag="et")
                        et = ett[:, :nj * 4]
                    ev = et.rearrange("p (j e) -> p j e", e=4)
                    nc.vector.memset(et, 0.0)
                    nc.vector.tensor_tensor(
                        out=ev[:, :, 0:3],
                        in0=g1[:, :nj * 6].rearrange("p (j e) -> p j e", e=6)[:, :, 0:3],
                        in1=g2[:, :nj * 6].rearrange("p (j e) -> p j e", e=6)[:, :, 3:6],
                        op=AT.add)
                    lrt = gp.tile([P, MNJ * 4], f32, tag="lrt")
                    nc.vector.tensor_scalar_min(lrt[:, :nj * 4], et, 0.0)
                    nc.vector.tensor_scalar_max(et, et, 0.0)
                    nc.vector.tensor_scalar_mul(lrt[:, :nj * 4], lrt[:, :nj * 4], NEG)
                    nc.vector.tensor_tensor(out=et, in0=et, in1=lrt[:, :nj * 4],
                                            op=AT.add)
                    nc.scalar.activation(ev[:, :, 0:3], ev[:, :, 0:3], AF.Exp)
                    nc.vector.tensor_tensor(out=et, in0=et,
                                            in1=mkc[:, :nj * 4], op=AT.mult)
                    ggg = nj // C
                    vb = lk[(j0, C)]
                    nc.vector.tensor_reduce(
                        out=sv[:, vb * 4:(vb + ggg) * 4].rearrange(
                            "p (g e) -> p g e", e=4),
                        in_=et.rearrange("p (g c e) -> p g e c", c=C, e=4),
                        axis=AX.X, op=AT.add)

            EXP = pp.tile([P, NTED * 4], bf16)
            SDt = pp.tile([P, NVD * 4], f32)
            nc.vector.memset(SDt[:], 0.0)
            if STAGE >= 2:
                score_pass(pland, lk_d, iSRCd, iDSTd, maskA, SDt, EXP)
            nc.sync.dma_start(
                SD.ap().rearrange("(p a) e -> p a e", p=P)[:, :, 0:4],
                SDt[:].rearrange("p (a e) -> p a e", e=4))

            SSt = pp.tile([P, NVS * 4], f32)
            nc.vector.memset(SSt[:], 0.0)
            if STAGE >= 3:
                score_pass(plans, lk_s, iSRCs, iDSTs, maskB, SSt, None)
            nc.sync.dma_start(
                SSh.ap().rearrange("(p a) e -> p a e", p=P),
                SSt[:].rearrange("p (a e) -> p a e", e=4))
            if STAGE >= 3:
                if USE_COLL:
                    nc.gpsimd.collective_compute(
                        "AllGather", AT.bypass, ins=[SSh.ap()], outs=[SS.ap()],
                        replica_groups=rg)
                else:
                    nc.sync.dma_start(SS.ap()[0:NSVS, :], SSh.ap())
                nc.sync.dma_start(SSL.ap()[:, 0:4], SS.ap())

            # ---- C: a = exp * sqrt(1/(sd*ss)) ----
            tc.strict_bb_all_engine_barrier()
            for (j0, nj, C) in (pland if STAGE >= 4 else []):
                i1 = idx_chunk(iSDv, j0, nj, "iel")
                i2 = idx_chunk(iSSv, j0, nj, "ier")
                g1 = gp.tile([P, MNJ * 4], f32, tag="g4a")
                g2 = gp.tile([P, MNJ * 4], f32, tag="g4b")
                if STAGE != 412:
                    nc.gpsimd.indirect_dma_start(
                        out=g1[:, :nj * 4], out_offset=None, in_=SD.ap(),
                        in_offset=IOA(ap=i1[:, :nj], axis=0))
                if STAGE != 411:
                    nc.gpsimd.indirect_dma_start(
                        out=g2[:, :nj * 4], out_offset=None, in_=SSL.ap(),
                        in_offset=IOA(ap=i2[:, :nj], axis=0))
                pr = g1[:, :nj * 4]
                if STAGE <= 8 or STAGE >= 42:
                    nc.vector.tensor_tensor(out=pr, in0=pr, in1=g2[:, :nj * 4],
                                            op=AT.mult)
                    nc.vector.tensor_scalar_max(pr, pr, 1e-30)
                if STAGE <= 8 or STAGE >= 43:
                    nc.vector.reciprocal(pr, pr)
                    nc.scalar.activation(pr, pr, AF.Sqrt)
                if STAGE >= 44 or STAGE <= 8:
                    nc.vector.tensor_tensor(
                        out=EXP[:, j0 * 4:(j0 + nj) * 4],
                        in0=EXP[:, j0 * 4:(j0 + nj) * 4], in1=pr, op=AT.mult)
            Av = EXP

            # ---- feat_trans ----
            def feat_trans(dst_ap, src_ap, k):
                nv = NVD
                m = wp.tile([P, nv * H], f32, tag="ftm")
                ms = wp.tile([P, nv * H], f32, tag="ftms")
                sv_ = src_ap.rearrange("p (a h d) -> p a h d", h=H, d=D)
                dv = dst_ap.rearrange("p (a h d) -> p a h d", h=H, d=D)
                mv = m[:].rearrange("p (a h) -> p a h", h=H)
                nc.vector.tensor_reduce(out=mv, in_=sv_, axis=AX.X, op=AT.add)
                nc.vector.tensor_scalar_mul(m[:], m[:], 1.0 / D)
                nc.scalar.activation(dst_ap, src_ap, AF.Square)
                nc.vector.tensor_reduce(
                    out=ms[:].rearrange("p (a h) -> p a h", h=H),
                    in_=dv, axis=AX.X, op=AT.add)
                nc.vector.tensor_scalar_mul(ms[:], ms[:], 1.0 / D)
                mm = wp.tile([P, nv * H], f32, tag="ftmm")
                nc.vector.tensor_tensor(out=mm[:], in0=m[:], in1=m[:],
                                        op=AT.mult)
                nc.vector.tensor_tensor(out=ms[:], in0=ms[:], in1=mm[:],
                                        op=AT.subtract)
                nc.scalar.activation(ms[:], ms[:], AF.Sqrt, bias=epst[:])
                nc.vector.reciprocal(ms[:], ms[:])
                mb = mv[:, :, :, None].to_broadcast([P, nv, H, D])
                rb = ms[:].rearrange("p (a h) -> p a h", h=H)[:, :, :, None] \
                    .to_broadcast([P, nv, H, D])
                nc.vector.tensor_tensor(out=dv, in0=sv_, in1=mb, op=AT.subtract)
                nc.vector.tensor_tensor(out=dv, in0=dv, in1=rb, op=AT.mult)
                nc.vector.tensor_tensor(
                    out=dv, in0=dv,
                    in1=sc4[:, k * HD:(k + 1) * HD].rearrange(
                        "p (h d) -> p h d", d=D)[:, None, :, :]
                    .to_broadcast([P, nv, H, D]), op=AT.mult)
                nc.vector.tensor_tensor(
                    out=dv, in0=dv,
                    in1=op4[:, k * HD:(k + 1) * HD].rearrange(
                        "p (h d) -> p h d", d=D)[:, None, :, :]
                    .to_broadcast([P, nv, H, D]), op=AT.add)

            # ---- h_query ----
            iq = pp.tile([P, NVD], i32)
            nc.sync.dma_start(iq[:], iQ.ap())
            CURV = pp.tile([P, NVD * HD], f32)
            HQ = pp.tile([P, NVD * HD], f32)
            if STAGE >= 5:
                nc.gpsimd.indirect_dma_start(
                    out=CURV[:].rearrange("p (a e) -> p a e", e=HD),
                    out_offset=None, in_=FTELER.ap(),
                    in_offset=IOA(ap=iq[:], axis=0))
            if STAGE >= 55 or (6 <= STAGE <= 8):
                feat_trans(HQ[:], CURV[:], 0)

            # ---- hops ----
            CURB = pp.tile([P, NVD * 64], bf16)
            nc.vector.memset(CURB[:], 0.0)
            KTOP = 0 if STAGE < 6 else (1 if STAGE == 6 else K)
            for k in range(1, KTOP + 1):
                tc.strict_bb_all_engine_barrier()
                nc.vector.memset(CURV[:], 0.0)
                for (j0, nj, C) in pland:
                    if k == 1:
                        isl = idx_chunk(iSRCd, j0, nj, "iel")
                        g = gp.tile([P, MNJ * HD], f32, tag="gh")
                        nc.gpsimd.indirect_dma_start(
                            out=g[:, :nj * HD].rearrange("p (a e) -> p a e", e=HD),
                            out_offset=None, in_=FTELER.ap(),
                            in_offset=IOA(ap=isl[:, :nj], axis=0))
                    else:
                        isl = idx_chunk(iCUR, j0, nj, "iel")
                        g = gp.tile([P, MNJ * HD], bf16, tag="gh")
                        nc.gpsimd.indirect_dma_start(
                            out=g[:, :nj * HD].rearrange("p (a e) -> p a e", e=HD),
                            out_offset=None, in_=CURGL[k - 2].ap(),
                            in_offset=IOA(ap=isl[:, :nj], axis=0))
                    gv = g[:, :nj * HD].rearrange(
                        "p (a h d) -> p a h d", h=H, d=D)
                    nc.vector.tensor_tensor(
                        out=gv, in0=gv,
                        in1=Av[:, j0 * 4:(j0 + nj) * 4].rearrange(
                            "p (a e) -> p a e", e=4)[:, :, 0:3][:, :, :, None]
                        .to_broadcast([P, nj, H, D]),
                        op=AT.mult)
                    ggg = nj // C
                    vb = lk_d[(j0, C)]
                    nc.vector.tensor_reduce(
                        out=CURV[:, vb * HD:(vb + ggg) * HD].rearrange(
                            "p (g e) -> p g e", e=HD),
                        in_=g[:, :nj * HD].rearrange(
                            "p (g c e) -> p g e c", c=C, e=HD),
                        axis=AX.X, op=AT.add)
                if k < K:
                    nc.vector.tensor_copy(
                        CURB[:].rearrange("p (a e) -> p a e", e=64)[:, :, 0:HD],
                        CURV[:].rearrange("p (a e) -> p a e", e=HD))
                    nc.sync.dma_start(
                        CURSH.ap().rearrange("(p a) e -> p a e", p=P),
                        CURB[:].rearrange("p (a e) -> p a e", e=64))
                    if USE_COLL:
                        nc.gpsimd.collective_compute(
                            "AllGather", AT.bypass, ins=[CURSH.ap()],
                            outs=[CURG[k - 1].ap()], replica_groups=rg)
                    else:
                        nc.sync.dma_start(CURG[k - 1].ap()[0:NSVD, :],
                                          CURSH.ap())
                    nc.sync.dma_start(CURGL[k - 1].ap(), CURG[k - 1].ap())
                HKt = wp.tile([P, NVD * HD], f32, tag="t0")
                feat_trans(HKt[:], CURV[:], k)
                nc.sync.dma_start(
                    HSTK[k - 1].ap().rearrange("(p a) e -> p a e", p=P),
                    HKt[:].rearrange("p (a e) -> p a e", e=HD))

            # ---- final hop attention ----
            if STAGE < 8:
                dum = wp.tile([P, NVD * HD], f32, tag="t0")
                nc.vector.memset(dum[:], 1.0)
                nc.sync.dma_start(
                    out_t.ap().rearrange("(p a) e -> p a e", p=P),
                    dum[:].rearrange("p (a e) -> p a e", e=HD))
            if STAGE >= 8:
                LG = wp.tile([P, NVD * H * 4], f32, tag="lg")
                nc.vector.memset(LG[:], 0.0)
                lgv = LG[:].rearrange("p (a h e) -> p a h e", h=H, e=4)[:, :, :, 0:K]
                lq = wp.tile([P, NVD * H], f32, tag="lq")
                t0 = wp.tile([P, NVD * HD], f32, tag="t0")
                hlv = hlr[:, 0:HD].rearrange("p (h d) -> p h d", d=D)[:, None, :, :] \
                    .to_broadcast([P, NVD, H, D])
                hrv = hlr[:, HD:2 * HD].rearrange("p (h d) -> p h d", d=D)[:, None, :, :] \
                    .to_broadcast([P, NVD, H, D])
                nc.vector.tensor_tensor(
                    out=t0[:].rearrange("p (a h d) -> p a h d", h=H, d=D),
                    in0=HQ[:].rearrange("p (a h d) -> p a h d", h=H, d=D),
                    in1=hlv, op=AT.mult)
                nc.vector.tensor_reduce(
                    out=lq[:].rearrange("p (a h) -> p a h", h=H),
                    in_=t0[:].rearrange("p (a h d) -> p a h d", h=H, d=D),
                    axis=AX.X, op=AT.add)
                wk = wp.tile([P, NVD * HD], f32, tag="wk")
                for k in range(K):
                    nc.sync.dma_start(
                        wk[:].rearrange("p (a e) -> p a e", e=HD),
                        HSTK[k].ap().rearrange("(p a) e -> p a e", p=P))
                    nc.vector.tensor_tensor(
                        out=t0[:].rearrange("p (a h d) -> p a h d", h=H, d=D),
                        in0=wk[:].rearrange("p (a h d) -> p a h d", h=H, d=D),
                        in1=hrv, op=AT.mult)
                    nc.vector.tensor_reduce(
                        out=LG[:].rearrange("p (a h e) -> p a h e", h=H, e=4)[:, :, :, k:k + 1],
                        in_=t0[:].rearrange("p (a h d) -> p a h d", h=H, d=D),
                        axis=AX.X, op=AT.add)
                nc.vector.tensor_tensor(
                    out=lgv, in0=lgv,
                    in1=lq[:].rearrange("p (a h) -> p a h", h=H)[:, :, :, None]
                    .to_broadcast([P, NVD, H, K]), op=AT.add)
                lrf = wp.tile([P, NVD * H * 4], f32, tag="lrf")
                nc.vector.tensor_scalar_min(lrf[:], LG[:], 0.0)
                nc.vector.tensor_scalar_max(LG[:], LG[:], 0.0)
                nc.vector.tensor_scalar_mul(lrf[:], lrf[:], NEG)
                nc.vector.tensor_tensor(out=LG[:], in0=LG[:], in1=lrf[:], op=AT.add)
                nc.scalar.activation(lgv, lgv, AF.Exp)
                den = wp.tile([P, NVD * H], f32, tag="den")
                nc.vector.tensor_reduce(
                    out=den[:].rearrange("p (a h) -> p a h", h=H),
                    in_=lgv, axis=AX.X, op=AT.add)
                nc.vector.reciprocal(den[:], den[:])
                acc = HQ
                nc.vector.memset(acc[:], 0.0)
                for k in range(K):
                    nc.sync.dma_start(
                        wk[:].rearrange("p (a e) -> p a e", e=HD),
                        HSTK[k].ap().rearrange("(p a) e -> p a e", p=P))
                    nc.vector.tensor_tensor(
                        out=t0[:].rearrange("p (a h d) -> p a h d", h=H, d=D),
                        in0=wk[:].rearrange("p (a h d) -> p a h d", h=H, d=D),
                        in1=LG[:].rearrange("p (a h e) -> p a h e", h=H, e=4)[:, :, :, k:k + 1]
                        .to_broadcast([P, NVD, H, D]), op=AT.mult)
                    nc.vector.tensor_tensor(out=acc[:], in0=acc[:], in1=t0[:],
                                            op=AT.add)
                nc.vector.tensor_tensor(
                    out=acc[:].rearrange("p (a h d) -> p a h d", h=H, d=D),
                    in0=acc[:].rearrange("p (a h d) -> p a h d", h=H, d=D),
                    in1=den[:].rearrange("p (a h) -> p a h", h=H)[:, :, :, None]
                    .to_broadcast([P, NVD, H, D]), op=AT.mult)
                nc.vector.tensor_tensor(
                    out=acc[:].rearrange("p (a e) -> p a e", e=HD),
                    in0=acc[:].rearrange("p (a e) -> p a e", e=HD),
                    in1=bia[:, None, :].to_broadcast([P, NVD, HD]), op=AT.add)
                nc.sync.dma_start(
                    out_t.ap().rearrange("(p a) e -> p a e", p=P),
                    acc[:].rearrange("p (a e) -> p a e", e=HD))
    nc.compile()
    return nc


# ---------------------------------------------------------------- entry
def kernel(**inputs):
    from concourse import bass_utils
    feat = np.asarray(inputs["feat"], np.float32)
    src = np.asarray(inputs["src"]).astype(np.int64)
    dst = np.asarray(inputs["dst"]).astype(np.int64)
    fc_W = np.asarray(inputs["fc_W"], np.float32)
    attn_l = np.asarray(inputs["attn_l"], np.float32).reshape(1, HD)
    attn_r = np.asarray(inputs["attn_r"], np.float32).reshape(1, HD)
    hop_l = np.asarray(inputs["hop_attn_l"], np.float32).reshape(1, HD)
    hop_r = np.asarray(inputs["hop_attn_r"], np.float32).reshape(1, HD)
    pos = np.asarray(inputs["pos_emb"], np.float32)
    nsc = np.asarray(inputs["norm_scales"], np.float32)
    off = np.asarray(inputs["offsets"], np.float32)
    bias = np.asarray(inputs["bias"], np.float32).reshape(1, HD)

    sides = host_prep(src, dst)
    sd, ssd = sides["d"], sides["s"]
    nc = build_nc(sd["NV"], sd["NTE"], sd["plan"],
                  ssd["NV"], ssd["NTE"], ssd["plan"])

    featP = np.zeros((NP_, IN), np.float32)
    featP[:N] = feat
    featT = np.ascontiguousarray(featP.T).astype(ml_dtypes.bfloat16)
    scales4 = np.tile(nsc.reshape(1, (K + 1) * HD), (P, 1))
    offpos4 = np.tile((off.reshape(K + 1, HD) +
                       pos[0].transpose(1, 0, 2).reshape(K + 1, H * D))
                      .reshape(1, -1), (P, 1))
    attn2 = np.tile(np.concatenate([attn_l, attn_r], 1), (P, 1))
    hop2 = np.tile(np.concatenate([hop_l, hop_r], 1), (P, 1))
    bias = np.tile(bias, (P, 1))

    in_maps = []
    for c in range(NCORES):
        cd, cs = sd["cores"][c], ssd["cores"][c]
        iq = np.full(P * sd["NV"], NP_, np.int64)
        iq[cd["vrow"]] = np.arange(NS) + c * NS
        in_maps.append(dict(
            featT=featT, W_in=fc_W, attn_lr=attn2, hop_lr=hop2,
            scales4=scales4, offpos4=offpos4, bias_in=bias,
            iSRCd=cd["ioth"], iDSTd=cd["ikey"],
            iSDv=(sd["vmap"][cd["ikey"]] % (P * sd["NV"])).astype(np.int32),
            iSSv=ssd["vmap"][cd["ioth"]].astype(np.int32),
            iCUR=sd["vmap"][cd["ioth"]].astype(np.int32),
            iSRCs=cs["ikey"], iDSTs=cs["ioth"],
            iQ=iq.reshape(P, sd["NV"]).astype(np.int32),
            maskA=cd["mask"].reshape(P, -1).astype(ml_dtypes.bfloat16),
            maskB=cs["mask"].reshape(P, -1).astype(ml_dtypes.bfloat16),
        ))
    outs = None
    if os.environ.get("AGDN_SIM", "") != "1":
        try:
            res = bass_utils.run_bass_kernel_spmd(
                nc, in_maps, core_ids=list(range(NCORES)))
            outs = [np.asarray(res.results[c]["out"]) for c in range(NCORES)]
            if not all(np.isfinite(o).all() for o in outs):
                print("[kernel] HW returned non-finite values", flush=True)
                outs = None
            else:
                print("[kernel] HW path OK", flush=True)
        except Exception as e:
            print(f"[kernel] HW path failed: {type(e).__name__}", flush=True)
            outs = None
    if outs is None:
        print("[kernel] falling back to MultiCoreSim", flush=True)
        # fall back to the cycle-accurate simulator (same Bass program)
        from concourse.bass_interp import MultiCoreSim
        sim = MultiCoreSim(nc, num_cores=NCORES, num_workers=NCORES,
                           trace=False, require_finite=False,
                           require_nnan=False)
        for c, core in enumerate(sim.cores.values()):
            for kk, vv in in_maps[c].items():
                core.tensor(kk)[:] = vv
        sim.simulate(check_with_hw=False)
        outs = [np.array(core.tensor("out")) for core in sim.cores.values()]
    full = np.concatenate(outs, 0)
    return full[sd["vmap"][:N]].reshape(N, H, D).astype(np.float32)

